# revision 12
# baseline (speedup 1.0000x reference)
"""Trainium2 Bass kernel for per-frame multi-head attention with partial RoPE.

Problem (hardcoded): b=2, N=4096, dim=512, H=8, DH=64, f=4 frames of n=1024
tokens, ROT_DIM=32 partial rotary, softmax attention per (b, h, frame) block,
then output projection.

Sharding: 8 cores = (batch, frame) pairs. Each core runs all 8 heads for one
1024-token frame -- fully independent, no collectives.

v4 design, software-pipelined around the ScalarE exp stream:
  - exp of the 8x [1024,1024] score blocks is the largest engine workload
    (~72us on ScalarE).  Per head-pair, the 16 S^T tiles are emitted at exp
    pace; all other PE work (previous pair's PV, deferred QKV chunks, V
    projection) is woven between them as filler batches.  Deferred-QKV /
    V-projection fillers pop with priority so their DVE tails (rope, copies)
    finish long before the next pair needs them.
  - PV stationary is [ones x64 | V] (M=128; matmul cost is driven by the
    moving operand, so the extra columns are free): PSUM rows 0-63 get the
    softmax denominator l replicated 64x, rows 64-127 the unnormalized O^T.
    Normalization per head = aligned PSUM->SBUF copy + in-place
    reciprocal_approx_fast + one tensor_mul (PSUM@64 x SBUF@0 verified on
    HW) -- no stream_shuffle, no cross-partition ops.
  - RoPE: partition pair-swap via stream_shuffle on a uint32 bitcast view
    (HW-verified; halves the shuffled element count), then fp16 cos/sin
    muls + add.
  - PE warm-up matmuls run during the DMA prologue so the HAM clock gate is
    at 8/8 when real work arrives.
  - Output projection writes fp16, halving the output DMA; host upcasts.

All matmul inputs are float16 (full PE rate; PSUM accumulation is fp32).
"""

from collections import deque
from contextlib import ExitStack

import numpy as np

import concourse.bass as bass
import concourse.tile as tile
from concourse import bacc
from concourse import mybir
from concourse.bass_utils import run_bass_kernel_spmd

F32 = mybir.dt.float32
U32 = mybir.dt.uint32
FP16 = mybir.dt.float16
MM_DT = FP16

B, N, DIM = 2, 4096, 512
H, DH = 8, 64
NF = 4                # frames
NTOK = 1024           # tokens per frame
ROT = 32
SCALE = DH ** -0.5
NCORES = 8

PAIRSWAP = [i ^ 1 for i in range(32)]


def build_program():
    """Build the single-core Bass/Tile program (SPMD across 8 cores)."""
    nc = bacc.Bacc(trn_type="TRN2", target_bir_lowering=False, debug=False)

    xt_d = nc.dram_tensor("xt", [DIM, NTOK], MM_DT, kind="ExternalInput").ap()
    wqkv_d = nc.dram_tensor("wqkv", [DIM, 3 * H * DH], MM_DT, kind="ExternalInput").ap()
    wout_d = nc.dram_tensor("wout", [H * DH, DIM], MM_DT, kind="ExternalInput").ap()
    bout_d = nc.dram_tensor("bout", [DIM], F32, kind="ExternalInput").ap()
    cosm_d = nc.dram_tensor("cosm", [128, NTOK], MM_DT, kind="ExternalInput").ap()
    sinm_d = nc.dram_tensor("sinm", [128, NTOK], MM_DT, kind="ExternalInput").ap()
    out_d = nc.dram_tensor("out_t", [DIM, NTOK], MM_DT, kind="ExternalOutput").ap()

    EXP = mybir.ActivationFunctionType.Exp

    with tile.TileContext(nc) as tc, ExitStack() as ctx:
        const = ctx.enter_context(tc.tile_pool(name="const", bufs=1))
        big = ctx.enter_context(tc.tile_pool(name="big", bufs=1))
        work = ctx.enter_context(tc.tile_pool(name="work", bufs=4))
        rlp = ctx.enter_context(tc.tile_pool(name="rlp", bufs=2))
        epool = ctx.enter_context(tc.tile_pool(name="E", bufs=30))
        psum = ctx.enter_context(tc.tile_pool(name="ps", bufs=2, space="PSUM"))

        # ---- DMAs ordered by first compute need:
        # k cols (S^T stationary chunk 4 first), x halves, q cols, cos/sin,
        # v cols, wout, bout.
        wqkv = const.tile([128, 4, 3 * H * DH], MM_DT, tag="wqkv", name="wqkv_sb")
        xT = big.tile([128, 4, NTOK], MM_DT, tag="xT", name="xT")
        xt_r = xt_d.rearrange("(kc p) t -> p kc t", p=128)
        wqkv_r = wqkv_d.rearrange("(kc p) c -> p kc c", p=128)
        nc.sync.dma_start(wqkv[:, :, 512:1024], wqkv_r[:, :, 512:1024])
        for tq in range(4):
            nc.sync.dma_start(xT[:, :, tq * 256:(tq + 1) * 256],
                              xt_r[:, :, tq * 256:(tq + 1) * 256])
        nc.sync.dma_start(wqkv[:, :, 0:512], wqkv_r[:, :, 0:512])
        cosm = const.tile([128, NTOK], MM_DT, tag="cosm", name="cosm_sb")
        nc.sync.dma_start(cosm[:], cosm_d)
        sinm = const.tile([128, NTOK], MM_DT, tag="sinm", name="sinm_sb")
        nc.sync.dma_start(sinm[:], sinm_d)
        nc.sync.dma_start(wqkv[:, :, 1024:1536], wqkv_r[:, :, 1024:1536])
        wout = const.tile([128, 4, DIM], MM_DT, tag="wout", name="wout_sb")
        nc.sync.dma_start(wout[:], wout_d.rearrange("(kc p) c -> p kc c", p=128))
        bout = const.tile([128, 4], F32, tag="bout", name="bout_sb")
        nc.sync.dma_start(bout[:], bout_d.rearrange("(c p) -> p c", p=128))

        # PE warm-up on scratch data while DMAs stream in (HAM -> 8/8)
        scr = const.tile([128, 512], MM_DT, tag="scr", name="scr_sb")
        nc.vector.memset(scr[:], 0.125)
        pwu = psum.tile([128, NTOK], F32, tag="ps", name="pwu")
        for _ in range(14):
            nc.tensor.matmul(pwu[:, 0:512], scr[:, 0:128], scr[:],
                             start=True, stop=True)

        # ---- big persistent buffers ----
        qsb = [big.tile([128, NTOK], MM_DT, tag=f"q{c}", name=f"q{c}") for c in range(4)]
        ksb = [big.tile([128, NTOK], MM_DT, tag=f"k{c}", name=f"k{c}") for c in range(4)]
        # V token-major per j-chunk: [128 tok, head, 128] with cols 0-63 all
        # ones (denominator broadcast) and cols 64-127 the V values.
        vsb = [big.tile([128, H, 128], MM_DT, tag=f"v{t}", name=f"v{t}") for t in range(8)]
        obar = [big.tile([128, NTOK], MM_DT, tag=f"ob{c}", name=f"ob{c}") for c in range(4)]
        outsb = [big.tile([128, NTOK], MM_DT, tag=f"os{c}", name=f"os{c}") for c in range(4)]

        for t in range(8):
            nc.vector.memset(vsb[t][:], 1.0)

        def rope(buf):
            tmp = work.tile([128, NTOK], MM_DT, tag="tmp", name="tmp")
            nc.vector.stream_shuffle(tmp[:].bitcast(U32), buf[:].bitcast(U32),
                                     PAIRSWAP)
            p1 = work.tile([128, NTOK], MM_DT, tag="tmp", name="tmp")
            nc.vector.tensor_mul(p1[:], buf[:], cosm[:])
            p2 = work.tile([128, NTOK], MM_DT, tag="tmp", name="tmp")
            nc.vector.tensor_mul(p2[:], tmp[:], sinm[:])
            nc.vector.tensor_add(buf[:], p1[:], p2[:])

        def emit_qkv_chunk(cc, copy_engine):
            """QKV projection for one 128-col q/k chunk + copy + RoPE."""
            dst = qsb[cc] if cc < 4 else ksb[cc - 4]
            pq = psum.tile([128, NTOK], F32, tag="ps", name="pq")
            for ih in range(2):
                for kc in range(4):
                    nc.tensor.matmul(
                        pq[:, ih * 512:(ih + 1) * 512],
                        wqkv[:, kc, cc * 128:(cc + 1) * 128],
                        xT[:, kc, ih * 512:(ih + 1) * 512],
                        start=(kc == 0), stop=(kc == 3),
                    )
            copy_engine(dst[:], pq[:])
            rope(dst)

        def emit_v_group(u):
            """V projection for token chunks 2u, 2u+1 (token-major)."""
            pv = psum.tile([128, NTOK], F32, tag="ps", name="pv")
            for tt in range(2):
                t = 2 * u + tt
                for kc in range(4):
                    nc.tensor.matmul(
                        pv[:, tt * 512:(tt + 1) * 512],
                        xT[:, kc, t * 128:(t + 1) * 128],
                        wqkv[:, kc, 1024:1536],
                        start=(kc == 0), stop=(kc == 3),
                    )
            for tt in range(2):
                t = 2 * u + tt
                nc.vector.tensor_copy(
                    vsb[t][:, :, 64:128],
                    pv[:, tt * 512:(tt + 1) * 512].rearrange(
                        "p (h d) -> p h d", h=H),
                )

        # ---- prologue: q/k chunk 0 so pair-0 attention can start ASAP ----
        emit_qkv_chunk(4, nc.scalar.copy)
        emit_qkv_chunk(0, nc.scalar.copy)

        # ---- filler machinery: prio (QKV/V, has DVE tails needed soon)
        # pops before normal (PV batches, tagged with their pair id).
        prio = deque()
        fillers = deque()
        pending = [0, 0, 0, 0]   # unpopped PV batches per pair

        def emit_pv_batch(pos_s, ets, pair, sub, jc):
            def go():
                for ih in range(2):
                    nc.tensor.matmul(
                        pos_s[:, ih * 512:(ih + 1) * 512],
                        vsb[jc][:, 2 * pair + sub, :],
                        ets[(sub, jc)][:, ih * 512:(ih + 1) * 512],
                        start=(jc == 0), stop=(jc == 7),
                    )
            return go

        def pop_fillers(k):
            # one prio closure counts as a full slot (8 MMs + DVE tail)
            if prio:
                prio.popleft()()
                return
            for _ in range(k):
                if fillers:
                    pid, go = fillers.popleft()
                    go()
                    pending[pid] -= 1

        prio.append(lambda: emit_qkv_chunk(5, nc.scalar.copy))
        prio.append(lambda: emit_qkv_chunk(1, nc.scalar.copy))
        for u in range(4):
            prio.append(lambda u=u: emit_v_group(u))

        def norm(pair, pos_pair):
            """obar[pair] = O^T / l.  pos rows 0-63 = l (x64), 64-127 = O^T."""
            for sub in range(2):
                pos_s = pos_pair[sub]
                lb = rlp.tile([64, NTOK], F32, tag=f"lb{sub}", name="lb")
                nc.vector.tensor_copy(lb[:], pos_s[0:64, :])
                nc.vector.reciprocal_approx_fast(lb[:], lb[:])
                nc.vector.tensor_mul(obar[pair][sub * 64:sub * 64 + 64, :],
                                     pos_s[64:128, :], lb[:])

        # ---- attention pair loop, software-pipelined ----
        pos_tiles = {}
        normed = [False] * 4
        for pair in range(4):
            qh = [qsb[pair][0:64, :], qsb[pair][64:128, :]]
            kh = [ksb[pair][0:64, :], ksb[pair][64:128, :]]
            # this pair's PV accumulators; first PE use is during pair+1
            pos_pair = (
                psum.tile([128, NTOK], F32, tag="pos", name="pos0"),
                psum.tile([128, NTOK], F32, tag="pos", name="pos1"),
            )
            pos_tiles[pair] = pos_pair
            ets = {}
            for jc in range(8):
                for sub in range(2):
                    ps = psum.tile([128, NTOK], F32, tag="ps", name="psw")
                    for ih in range(2):
                        nc.tensor.matmul(
                            ps[:, ih * 512:(ih + 1) * 512],
                            kh[sub][:, jc * 128:(jc + 1) * 128],
                            qh[sub][:, ih * 512:(ih + 1) * 512],
                            start=True, stop=True,
                            tile_position=(sub * 64, 0),
                        )
                    et = epool.tile([128, NTOK], MM_DT, tag="E", name="et")
                    ets[(sub, jc)] = et
                    nc.scalar.activation(et[:], ps[:], EXP)
                    fillers.append(
                        (pair, emit_pv_batch(pos_pair[sub], ets, pair, sub, jc)))
                    pending[pair] += 1
                # pair 0: let the exp stream establish before inserting work
                if pair == 0 and jc < 2:
                    continue
                pop_fillers(2 if pair < 3 else 3)
                # mid-pair-3: as soon as PV(2) drains, norm it so PV(3)
                # (queued behind) can take over the freed pos buffers
                if pair == 3 and not normed[2] and pending[2] == 0:
                    norm(2, pos_tiles[2])
                    normed[2] = True
            if pair >= 1:
                while pending[pair - 1] > 0 or prio:
                    pop_fillers(2)
                if not normed[pair - 1]:
                    norm(pair - 1, pos_tiles[pair - 1])
                    normed[pair - 1] = True

            if pair < 2:
                prio.append(
                    lambda cc=pair + 6: emit_qkv_chunk(cc, nc.vector.tensor_copy))
                prio.append(
                    lambda cc=pair + 2: emit_qkv_chunk(cc, nc.vector.tensor_copy))

        # ---- tail: rest of PV(3), out-proj partials, norm(3), finish ----
        while fillers:
            pop_fillers(2)

        def outproj_mms(oc, fcs):
            for ih in range(2):
                for fc in fcs:
                    nc.tensor.matmul(
                        pf_tiles[oc][:, ih * 512:(ih + 1) * 512],
                        wout[:, fc, oc * 128:(oc + 1) * 128],
                        obar[fc][:, ih * 512:(ih + 1) * 512],
                        start=(fc == 0), stop=(fc == 3),
                    )

        # oc0/oc1 partials (fc 0-2) don't need obar[3]; run before norm(3)
        pf_tiles = {oc: psum.tile([128, NTOK], F32, tag="ps", name="pf")
                    for oc in range(2)}
        outproj_mms(0, range(3))
        outproj_mms(1, range(3))
        norm(3, pos_tiles[3])
        for oc in range(4):
            if oc >= 2:
                pf_tiles[oc] = psum.tile([128, NTOK], F32, tag="ps", name="pf")
                outproj_mms(oc, range(4))
            else:
                outproj_mms(oc, [3])
            nc.vector.tensor_scalar_add(outsb[oc][:], pf_tiles[oc][:],
                                        bout[:, oc:oc + 1])
            nc.sync.dma_start(out_d[oc * 128:(oc + 1) * 128, :], outsb[oc][:])

    nc.compile()
    return nc


def host_prep(x, W_qkv, W_out, b_out, sin, cos):
    """Build the per-core input tensors (host-side prep, incl. x transpose)."""
    x = np.asarray(x, dtype=np.float32)
    W_qkv = np.asarray(W_qkv, dtype=np.float32).copy()
    W_out = np.ascontiguousarray(np.asarray(W_out, dtype=np.float32))
    b_out = np.ascontiguousarray(np.asarray(b_out, dtype=np.float32))
    sin = np.asarray(sin, dtype=np.float32)
    cos = np.asarray(cos, dtype=np.float32)

    # fold q scaling into W_qkv's q block
    W_qkv[:, 0:H * DH] *= SCALE

    # masked, feature-major cos/sin tiles [128, 1024]
    dloc = np.arange(128) % DH
    sign = np.where(np.arange(128) % 2 == 0, -1.0, 1.0).astype(np.float32)
    cosT = cos.T.astype(np.float32)  # [32, 1024]
    sinT = sin.T.astype(np.float32)
    cosm = np.ones((128, NTOK), dtype=np.float32)
    sinm = np.zeros((128, NTOK), dtype=np.float32)
    rot_rows = dloc < ROT
    cosm[rot_rows] = cosT[dloc[rot_rows]]
    sinm[rot_rows] = sinT[dloc[rot_rows]] * sign[rot_rows][:, None]

    shared = {
        "wqkv": W_qkv.astype(np.float16), "wout": W_out.astype(np.float16),
        "bout": b_out, "cosm": cosm.astype(np.float16),
        "sinm": sinm.astype(np.float16),
    }
    in_maps = []
    for c in range(NCORES):
        bi, fi = c // NF, c % NF
        m = dict(shared)
        m["xt"] = np.ascontiguousarray(x[bi, fi * NTOK:(fi + 1) * NTOK, :].T).astype(np.float16)
        in_maps.append(m)
    return in_maps


_CACHED_NC = None


def kernel(x, W_qkv, W_out, b_out, sin, cos, f=4, **run_kwargs):
    global _CACHED_NC
    assert int(f) == NF
    in_maps = host_prep(x, W_qkv, W_out, b_out, sin, cos)
    if _CACHED_NC is None:
        _CACHED_NC = build_program()
    res = run_bass_kernel_spmd(
        _CACHED_NC, in_maps, core_ids=list(range(NCORES)), **run_kwargs
    )
    out = np.empty((B, N, DIM), dtype=np.float32)
    for c in range(NCORES):
        bi, fi = c // NF, c % NF
        out[bi, fi * NTOK:(fi + 1) * NTOK, :] = res.results[c]["out_t"].T.astype(np.float32)
    if run_kwargs:
        kernel.last_results = res
    return out


# revision 16
# speedup vs baseline: 1.0987x; 1.0987x over previous
"""Trainium2 Bass kernel for per-frame multi-head attention with partial RoPE.

Problem (hardcoded): b=2, N=4096, dim=512, H=8, DH=64, f=4 frames of n=1024
tokens, ROT_DIM=32 partial rotary, softmax attention per (b, h, frame) block,
then output projection.

Sharding: 8 cores = (batch, frame) pairs. Each core runs all 8 heads for one
1024-token frame -- fully independent, no collectives.

v4 design, software-pipelined around the ScalarE exp stream:
  - exp of the 8x [1024,1024] score blocks is the largest engine workload
    (~72us on ScalarE).  Per head-pair, the 16 S^T tiles are emitted at exp
    pace; all other PE work (previous pair's PV, deferred QKV chunks, V
    projection) is woven between them as filler batches.  Deferred-QKV /
    V-projection fillers pop with priority so their DVE tails (rope, copies)
    finish long before the next pair needs them.
  - PV stationary is [ones x64 | V] (M=128; matmul cost is driven by the
    moving operand, so the extra columns are free): PSUM rows 0-63 get the
    softmax denominator l replicated 64x, rows 64-127 the unnormalized O^T.
    Normalization per head = aligned PSUM->SBUF copy + in-place
    reciprocal_approx_fast + one tensor_mul (PSUM@64 x SBUF@0 verified on
    HW) -- no stream_shuffle, no cross-partition ops.
  - RoPE: partition pair-swap via stream_shuffle on a uint32 bitcast view
    (HW-verified; halves the shuffled element count), then fp16 cos/sin
    muls + add.
  - PE warm-up matmuls run during the DMA prologue so the HAM clock gate is
    at 8/8 when real work arrives.
  - Output projection writes fp16, halving the output DMA; host upcasts.

All matmul inputs are float16 (full PE rate; PSUM accumulation is fp32).
"""

from collections import deque
from contextlib import ExitStack

import numpy as np

import concourse.bass as bass
import concourse.tile as tile
from concourse import bacc
from concourse import mybir
from concourse.bass_utils import run_bass_kernel_spmd

F32 = mybir.dt.float32
U32 = mybir.dt.uint32
FP16 = mybir.dt.float16
MM_DT = FP16

B, N, DIM = 2, 4096, 512
H, DH = 8, 64
NF = 4                # frames
NTOK = 1024           # tokens per frame
ROT = 32
SCALE = DH ** -0.5
NCORES = 8

PAIRSWAP = [i ^ 1 for i in range(32)]


def build_program():
    """Build the single-core Bass/Tile program (SPMD across 8 cores)."""
    nc = bacc.Bacc(trn_type="TRN2", target_bir_lowering=False, debug=False)

    xt_d = nc.dram_tensor("xt", [DIM, NTOK], MM_DT, kind="ExternalInput").ap()
    wqkv_d = nc.dram_tensor("wqkv", [DIM, 3 * H * DH], MM_DT, kind="ExternalInput").ap()
    wout_d = nc.dram_tensor("wout", [H * DH, DIM], MM_DT, kind="ExternalInput").ap()
    bout_d = nc.dram_tensor("bout", [DIM], F32, kind="ExternalInput").ap()
    cosm_d = nc.dram_tensor("cosm", [128, NTOK], MM_DT, kind="ExternalInput").ap()
    sinm_d = nc.dram_tensor("sinm", [128, NTOK], MM_DT, kind="ExternalInput").ap()
    out_d = nc.dram_tensor("out_t", [DIM, NTOK], MM_DT, kind="ExternalOutput").ap()

    EXP = mybir.ActivationFunctionType.Exp

    with tile.TileContext(nc) as tc, ExitStack() as ctx:
        const = ctx.enter_context(tc.tile_pool(name="const", bufs=1))
        big = ctx.enter_context(tc.tile_pool(name="big", bufs=1))
        work = ctx.enter_context(tc.tile_pool(name="work", bufs=4))
        rlp = ctx.enter_context(tc.tile_pool(name="rlp", bufs=2))
        epool = ctx.enter_context(tc.tile_pool(name="E", bufs=30))
        psum = ctx.enter_context(tc.tile_pool(name="ps", bufs=2, space="PSUM"))

        # ---- DMAs ordered by first compute need:
        # k cols (S^T stationary chunk 4 first), x halves, q cols, cos/sin,
        # v cols, wout, bout.
        wqkv = const.tile([128, 4, 3 * H * DH], MM_DT, tag="wqkv", name="wqkv_sb")
        xT = big.tile([128, 4, NTOK], MM_DT, tag="xT", name="xT")
        xt_r = xt_d.rearrange("(kc p) t -> p kc t", p=128)
        wqkv_r = wqkv_d.rearrange("(kc p) c -> p kc c", p=128)
        nc.sync.dma_start(wqkv[:, :, 512:1024], wqkv_r[:, :, 512:1024])
        for tq in range(4):
            nc.sync.dma_start(xT[:, :, tq * 256:(tq + 1) * 256],
                              xt_r[:, :, tq * 256:(tq + 1) * 256])
        nc.sync.dma_start(wqkv[:, :, 0:512], wqkv_r[:, :, 0:512])
        cosm = const.tile([128, NTOK], MM_DT, tag="cosm", name="cosm_sb")
        nc.sync.dma_start(cosm[:], cosm_d)
        sinm = const.tile([128, NTOK], MM_DT, tag="sinm", name="sinm_sb")
        nc.sync.dma_start(sinm[:], sinm_d)
        nc.sync.dma_start(wqkv[:, :, 1024:1536], wqkv_r[:, :, 1024:1536])
        wout = const.tile([128, 4, DIM], MM_DT, tag="wout", name="wout_sb")
        nc.sync.dma_start(wout[:], wout_d.rearrange("(kc p) c -> p kc c", p=128))
        bout = const.tile([128, 4], F32, tag="bout", name="bout_sb")
        nc.sync.dma_start(bout[:], bout_d.rearrange("(c p) -> p c", p=128))

        # PE warm-up on scratch data while DMAs stream in (HAM -> 8/8)
        scr = const.tile([128, 512], MM_DT, tag="scr", name="scr_sb")
        nc.vector.memset(scr[:], 0.125)
        pwu = psum.tile([128, NTOK], F32, tag="ps", name="pwu")
        for _ in range(14):
            nc.tensor.matmul(pwu[:, 0:512], scr[:, 0:128], scr[:],
                             start=True, stop=True)

        # ---- big persistent buffers ----
        qsb = [big.tile([128, NTOK], MM_DT, tag=f"q{c}", name=f"q{c}") for c in range(4)]
        ksb = [big.tile([128, NTOK], MM_DT, tag=f"k{c}", name=f"k{c}") for c in range(4)]
        # V token-major per j-chunk: [128 tok, head, 128] with cols 0-63 all
        # ones (denominator broadcast) and cols 64-127 the V values.
        vsb = [big.tile([128, H, 128], MM_DT, tag=f"v{t}", name=f"v{t}") for t in range(8)]
        obar = [big.tile([128, NTOK], MM_DT, tag=f"ob{c}", name=f"ob{c}") for c in range(4)]
        outsb = [big.tile([128, NTOK], MM_DT, tag=f"os{c}", name=f"os{c}") for c in range(4)]

        for t in range(8):
            nc.gpsimd.memset(vsb[t][:], 1.0)

        def rope(buf):
            tmp = work.tile([128, NTOK], MM_DT, tag="tmp", name="tmp")
            nc.vector.stream_shuffle(tmp[:].bitcast(U32), buf[:].bitcast(U32),
                                     PAIRSWAP)
            p1 = work.tile([128, NTOK], MM_DT, tag="tmp", name="tmp")
            nc.vector.tensor_mul(p1[:], buf[:], cosm[:])
            p2 = work.tile([128, NTOK], MM_DT, tag="tmp", name="tmp")
            nc.vector.tensor_mul(p2[:], tmp[:], sinm[:])
            nc.vector.tensor_add(buf[:], p1[:], p2[:])

        def emit_qkv_chunk(cc, copy_engine):
            """QKV projection for one 128-col q/k chunk + copy + RoPE."""
            dst = qsb[cc] if cc < 4 else ksb[cc - 4]
            pq = psum.tile([128, NTOK], F32, tag="ps", name="pq")
            for ih in range(2):
                for kc in range(4):
                    nc.tensor.matmul(
                        pq[:, ih * 512:(ih + 1) * 512],
                        wqkv[:, kc, cc * 128:(cc + 1) * 128],
                        xT[:, kc, ih * 512:(ih + 1) * 512],
                        start=(kc == 0), stop=(kc == 3),
                    )
            copy_engine(dst[:], pq[:])
            rope(dst)

        def emit_v_group(u):
            """V projection for token chunks 2u, 2u+1 (token-major)."""
            pv = psum.tile([128, NTOK], F32, tag="ps", name="pv")
            for tt in range(2):
                t = 2 * u + tt
                for kc in range(4):
                    nc.tensor.matmul(
                        pv[:, tt * 512:(tt + 1) * 512],
                        xT[:, kc, t * 128:(t + 1) * 128],
                        wqkv[:, kc, 1024:1536],
                        start=(kc == 0), stop=(kc == 3),
                    )
            for tt in range(2):
                t = 2 * u + tt
                nc.vector.tensor_copy(
                    vsb[t][:, :, 64:128],
                    pv[:, tt * 512:(tt + 1) * 512].rearrange(
                        "p (h d) -> p h d", h=H),
                )

        # ---- prologue: q/k chunk 0 so pair-0 attention can start ASAP ----
        emit_qkv_chunk(4, nc.scalar.copy)
        emit_qkv_chunk(0, nc.scalar.copy)

        # ---- filler machinery: prio (QKV/V, has DVE tails needed soon)
        # pops before normal (PV batches, tagged with their pair id).
        prio = deque()
        fillers = deque()
        pending = [0, 0, 0, 0]   # unpopped PV batches per pair

        def emit_pv_batch(pos_s, ets, pair, sub, jc):
            def go():
                for ih in range(2):
                    nc.tensor.matmul(
                        pos_s[:, ih * 512:(ih + 1) * 512],
                        vsb[jc][:, 2 * pair + sub, :],
                        ets[(sub, jc)][:, ih * 512:(ih + 1) * 512],
                        start=(jc == 0), stop=(jc == 7),
                    )
            return go

        def pop_fillers(k):
            # one prio closure counts as a full slot (8 MMs + DVE tail)
            if prio:
                prio.popleft()()
                return
            for _ in range(k):
                if fillers:
                    pid, go = fillers.popleft()
                    go()
                    pending[pid] -= 1

        prio.append(lambda: emit_qkv_chunk(5, nc.scalar.copy))
        prio.append(lambda: emit_qkv_chunk(1, nc.scalar.copy))
        for u in range(4):
            prio.append(lambda u=u: emit_v_group(u))

        def norm(pair, pos_pair):
            """obar[pair] = O^T / l.  pos rows 0-63 = l (x64), 64-127 = O^T."""
            for sub in range(2):
                pos_s = pos_pair[sub]
                lb = rlp.tile([64, NTOK], F32, tag=f"lb{sub}", name="lb")
                nc.vector.tensor_copy(lb[:], pos_s[0:64, :])
                nc.vector.reciprocal_approx_fast(lb[:], lb[:])
                nc.vector.tensor_mul(obar[pair][sub * 64:sub * 64 + 64, :],
                                     pos_s[64:128, :], lb[:])

        # ---- attention pair loop, software-pipelined ----
        pos_tiles = {}
        normed = [False] * 4
        for pair in range(4):
            qh = [qsb[pair][0:64, :], qsb[pair][64:128, :]]
            kh = [ksb[pair][0:64, :], ksb[pair][64:128, :]]
            # this pair's PV accumulators; first PE use is during pair+1
            pos_pair = (
                psum.tile([128, NTOK], F32, tag="pos", name="pos0"),
                psum.tile([128, NTOK], F32, tag="pos", name="pos1"),
            )
            pos_tiles[pair] = pos_pair
            ets = {}
            for jc in range(8):
                for sub in range(2):
                    ps = psum.tile([128, NTOK], F32, tag="ps", name="psw")
                    for ih in range(2):
                        nc.tensor.matmul(
                            ps[:, ih * 512:(ih + 1) * 512],
                            kh[sub][:, jc * 128:(jc + 1) * 128],
                            qh[sub][:, ih * 512:(ih + 1) * 512],
                            start=True, stop=True,
                            tile_position=(sub * 64, 0),
                        )
                    et = epool.tile([128, NTOK], MM_DT, tag="E", name="et")
                    ets[(sub, jc)] = et
                    nc.scalar.activation(et[:], ps[:], EXP)
                    fillers.append(
                        (pair, emit_pv_batch(pos_pair[sub], ets, pair, sub, jc)))
                    pending[pair] += 1
                # pair 0: let the exp stream establish before inserting work
                if pair == 0 and jc < 2:
                    continue
                pop_fillers(3)
                # as soon as PV(pair-1) drains, norm it mid-pair: frees the
                # pos buffers so PV(pair) can flow from the next pair's start
                if pair >= 1 and not normed[pair - 1] and pending[pair - 1] == 0:
                    norm(pair - 1, pos_tiles[pair - 1])
                    normed[pair - 1] = True
            if pair >= 1 and not normed[pair - 1]:
                while pending[pair - 1] > 0 or prio:
                    pop_fillers(2)
                norm(pair - 1, pos_tiles[pair - 1])
                normed[pair - 1] = True

            if pair < 2:
                prio.append(
                    lambda cc=pair + 6: emit_qkv_chunk(cc, nc.vector.tensor_copy))
                prio.append(
                    lambda cc=pair + 2: emit_qkv_chunk(cc, nc.vector.tensor_copy))

        # ---- tail: rest of PV(3), out-proj partials, norm(3), finish ----
        while fillers:
            pop_fillers(2)

        def outproj_mms(oc, fcs):
            for ih in range(2):
                for fc in fcs:
                    nc.tensor.matmul(
                        pf_tiles[oc][:, ih * 512:(ih + 1) * 512],
                        wout[:, fc, oc * 128:(oc + 1) * 128],
                        obar[fc][:, ih * 512:(ih + 1) * 512],
                        start=(fc == 0), stop=(fc == 3),
                    )

        # oc0/oc1 partials (fc 0-2) don't need obar[3]; run before norm(3)
        pf_tiles = {oc: psum.tile([128, NTOK], F32, tag="ps", name="pf")
                    for oc in range(2)}
        outproj_mms(0, range(3))
        outproj_mms(1, range(3))
        norm(3, pos_tiles[3])
        # keep the PE warm through norm(3)'s DVE chain so the remaining
        # projection matmuls don't run at the throttled clock (LDWEIGHTS
        # counts as PE activity and touches no PSUM)
        for _ in range(30):
            nc.tensor.ldweights(scr[:, 0:128])
        for oc in range(4):
            if oc >= 2:
                pf_tiles[oc] = psum.tile([128, NTOK], F32, tag="ps", name="pf")
                outproj_mms(oc, range(4))
            else:
                outproj_mms(oc, [3])
            nc.scalar.add(outsb[oc][:], pf_tiles[oc][:], bout[:, oc:oc + 1])
            nc.sync.dma_start(out_d[oc * 128:(oc + 1) * 128, :], outsb[oc][:])

    nc.compile()
    return nc


def host_prep(x, W_qkv, W_out, b_out, sin, cos):
    """Build the per-core input tensors (host-side prep, incl. x transpose)."""
    x = np.asarray(x, dtype=np.float32)
    W_qkv = np.asarray(W_qkv, dtype=np.float32).copy()
    W_out = np.ascontiguousarray(np.asarray(W_out, dtype=np.float32))
    b_out = np.ascontiguousarray(np.asarray(b_out, dtype=np.float32))
    sin = np.asarray(sin, dtype=np.float32)
    cos = np.asarray(cos, dtype=np.float32)

    # fold q scaling into W_qkv's q block
    W_qkv[:, 0:H * DH] *= SCALE

    # masked, feature-major cos/sin tiles [128, 1024]
    dloc = np.arange(128) % DH
    sign = np.where(np.arange(128) % 2 == 0, -1.0, 1.0).astype(np.float32)
    cosT = cos.T.astype(np.float32)  # [32, 1024]
    sinT = sin.T.astype(np.float32)
    cosm = np.ones((128, NTOK), dtype=np.float32)
    sinm = np.zeros((128, NTOK), dtype=np.float32)
    rot_rows = dloc < ROT
    cosm[rot_rows] = cosT[dloc[rot_rows]]
    sinm[rot_rows] = sinT[dloc[rot_rows]] * sign[rot_rows][:, None]

    shared = {
        "wqkv": W_qkv.astype(np.float16), "wout": W_out.astype(np.float16),
        "bout": b_out, "cosm": cosm.astype(np.float16),
        "sinm": sinm.astype(np.float16),
    }
    in_maps = []
    for c in range(NCORES):
        bi, fi = c // NF, c % NF
        m = dict(shared)
        m["xt"] = np.ascontiguousarray(x[bi, fi * NTOK:(fi + 1) * NTOK, :].T).astype(np.float16)
        in_maps.append(m)
    return in_maps


_CACHED_NC = None


def kernel(x, W_qkv, W_out, b_out, sin, cos, f=4, **run_kwargs):
    global _CACHED_NC
    assert int(f) == NF
    in_maps = host_prep(x, W_qkv, W_out, b_out, sin, cos)
    if _CACHED_NC is None:
        _CACHED_NC = build_program()
    res = run_bass_kernel_spmd(
        _CACHED_NC, in_maps, core_ids=list(range(NCORES)), **run_kwargs
    )
    out = np.empty((B, N, DIM), dtype=np.float32)
    for c in range(NCORES):
        bi, fi = c // NF, c % NF
        out[bi, fi * NTOK:(fi + 1) * NTOK, :] = res.results[c]["out_t"].T.astype(np.float32)
    if run_kwargs:
        kernel.last_results = res
    return out
